# revision 134
# baseline (speedup 1.0000x reference)
"""Trainium2 Bass kernel for a Transformer-XL (MemTransformerLM) layer.

Sharding over 8 cores: core c = (b = c//4, head-group g = c%4 of 4 heads).
Each core computes its 4 heads' attention for its batch, a partial
attn_out = vec @ W_o[:, heads].T, then a ReduceScatter(+) over the quad
[[0..3],[4..7]] scatters query rows -> each core does LN1+FF+LN2 on its
256 rows. Host reassembles [1024, 2, 1024].

rel_shift: B = q_tilde @ rk^T goes to f32 PSUM, staged to SBUF bf16
(band-trimmed: q-tile qt only writes columns >= 896-128*qt), written per
q-tile to DRAM with row stride 2176; BD[i,j] = B[i, j-i+1023] is read
back with batched oblique APs (one [128, 4x512] read per 512-key block
covering 4 q-tiles), cast to f32 on DVE/Pool, and PE-transpose-
accumulated (f32, 2 cyc/row) into the AC^T PSUM group, so S^T = AC^T +
BD^T lands in PSUM with no extra vector pass. Mask: pad columns
[2048, 2176) are -1e30 so the band tile kt == qt+8 picks up NEG in its
upper triangle; tiles kt > qt+8 are skipped. Softmax skips the
max-subtraction; the denominator comes free from a ones-column in v.

LN1's affine is folded into the FF weights host-side (W1' = W1*g1,
b1' = b1 + W1@ln1_b, b2' = b2 + ln1_b). LN stats come from DVE
bn_stats/bn_aggr; z*g1 is injected into the FF2 PSUM group by an
identity matmul so LN2 reads stats/z straight from PSUM. The FF runs as
two row-tile sweeps (t=0 first, so ReduceScatter #2 hides under it; the
t=1 sweep walks w1/w2 blocks in reverse so the freshest blocks are
still resident), with FF2 trailing FF1 by one mi to hide relu latency.

Queue/engine discipline (the scheduling model serializes each DMA queue
and parks it on the first waiting transfer):
- SP: input prefetch stream + oblique reads (self-pacing WAR parks)
- Act HWDGE: B writes, attn_part writes, scores-time weight chunks
  (512KB chunks so they never monopolize the shared DMA pool)
- Pool SWDGE: collectives + rsx readbacks ONLY (a collective holds its
  SEQ while waiting, so nothing else may queue behind it)
- bd casts: DVE (+Pool for the first half, whose queue is still clear);
  exp is the Act-engine pacer so everything else avoids Act during the
  second half. Act Sqrt (LN rstd) is emitted only after the attn_part
  writes so its table switch cannot delay the ReduceScatter.
"""
import functools
from contextlib import ExitStack
import numpy as np

QLEN, MLEN, BSZ = 1024, 1024, 2
KLEN = QLEN + MLEN
D, H, DH, DI = 1024, 16, 64, 4096
HPG = 4                      # heads per group (per core)
HD_G = HPG * DH              # 256
N_CORES = 8
SCALE = 1.0 / (DH ** 0.5)
NEG = -1e30
BW = 2176                    # padded DRAM width for B (>= 2175)
NQT = QLEN // 128            # 8 query tiles of 128
NKT = KLEN // 128            # 16 key tiles of 128
NDC = D // 128               # 8 d-chunks
NMI = DI // 128              # 32 inner tiles
ROWS = QLEN // 4             # 256 rows per core after RS


@functools.lru_cache(maxsize=2)
def _build(single_sim=False):
    import concourse.bacc as bacc
    import concourse.mybir as mybir
    import concourse.tile as tile
    from concourse import masks
    import bass_rust

    F32 = mybir.dt.float32
    BF16 = mybir.dt.bfloat16
    AF = mybir.ActivationFunctionType
    ALU = mybir.AluOpType

    nc = bacc.Bacc("TRN2", target_bir_lowering=False, debug=False,
                   num_devices=N_CORES)

    def din(name, shape, dt=F32):
        return nc.dram_tensor(name, shape, dt, kind="ExternalInput")

    cat_fm = din("cat_fm", [D, KLEN], BF16)   # [d, mems||w tokens], this b
    r_fm = din("r_fm", [D, KLEN], BF16)       # r transposed
    wpk = din("wpk", [D, 4 * HD_G], BF16)     # [wkT | wqT | wrT | wvT]
    biases = din("biases", [128, 8])          # raw rwb/rrb + pre-scaled
    woT = din("woT", [HD_G, D], BF16)         # W_o^T rows for group
    w1T = din("w1T", [D, DI], BF16)           # (W1 * g1)^T
    b1c = din("b1c", [128, NMI])              # b1' packed column-wise
    w2T = din("w2T", [DI, D], BF16)
    b2s = din("b2s", [1, D], BF16)            # b2 + ln1_b, single row
    rows4 = din("rows4", [128, 3 * D], BF16)  # [g1 | g2 | ln2_b] bcast rows
    wres2 = din("wres2", [128, 2 * D], BF16)  # w rows for residual (2 tiles)

    Bh = [nc.dram_tensor(f"Bh{h}", [QLEN * BW], BF16) for h in range(HPG)]
    if single_sim:
        attn_part = nc.dram_tensor("attn_part", [QLEN, D], BF16,
                                   kind="ExternalOutput")
    else:
        attn_part = nc.dram_tensor("attn_part", [QLEN, D], BF16)
    rs_out = nc.dram_tensor("rs_out", [ROWS, D], BF16)
    y = nc.dram_tensor("y", [ROWS, D], F32, kind="ExternalOutput")

    def obl(h, qh2, ktb):
        # oblique view of Bh[h]: BD tiles [query, key] for 4 q-tiles x 512
        # keys at (q-tiles 4*qh2.., keys 512*ktb..); BD[i,j] = B[i, j-i+1023];
        # matches dest [128, 4, 512]
        off = 1023 + 512 * qh2 * (BW - 1) + 512 * ktb
        return bass_rust.AP(tensor=Bh[h].ap().tensor, offset=off,
                            ap=[[BW - 1, 128], [128 * (BW - 1), 4], [1, 512]])

    def bwrite(h, qt, c0, w):
        # B row tile [128, w] at (row 128*qt, col c0)
        off = 128 * qt * BW + c0
        return bass_rust.AP(tensor=Bh[h].ap().tensor, offset=off,
                            ap=[[BW, 128], [1, w]])

    def bpad(h):
        # all pad columns of head h as one flat write; src is a [128, 128]
        # NEG tile re-read NQT times via a stride-0 leading dim
        off = 2048
        return bass_rust.AP(tensor=Bh[h].ap().tensor, offset=off,
                            ap=[[128 * BW, NQT], [BW, 128], [1, BW - 2048]])

    with tile.TileContext(nc) as tc:
        with tc.tile_pool(name="const", bufs=1) as cpool, \
             tc.tile_pool(name="work", bufs=2) as wpool, \
             tc.tile_pool(name="psA", bufs=2, space="PSUM") as psA, \
             tc.tile_pool(name="psB", bufs=2, space="PSUM") as psB, \
             tc.tile_pool(name="psBB", bufs=2, space="PSUM") as psBB, \
             tc.tile_pool(name="psVT", bufs=2, space="PSUM") as psVT:
            w1pool = w2pool = None  # opened after the P1 scope closes

            # ---------------- global constants ----------------
            identb = cpool.tile([128, 128], BF16, tag="identb")
            masks.make_identity(nc, identb[:])
            ident = cpool.tile([128, 128], F32, tag="ident")
            masks.make_identity(nc, ident[:])
            bias_t = cpool.tile([128, 8], F32, tag="bias")
            nc.scalar.dma_start(out=bias_t[:], in_=biases[:])
            b1c_t = cpool.tile([128, NMI], F32, tag="b1c")
            nc.scalar.dma_start(out=b1c_t[:], in_=b1c[:])
            # z (normalized LN1 output) survives into FF scope
            zt = [cpool.tile([128, D], BF16, tag=f"zt_{t}", name=f"zt_{t}")
                  for t in range(ROWS // 128)]
            out1_fm = [None] * NDC
            zgs = [None] * (ROWS // 128)

            onesr = cpool.tile([1, 128], BF16, tag="onesr")
            nc.vector.memset(onesr[:], 1.0)
            b2b = cpool.tile([1, D], BF16, tag="b2b")
            nc.scalar.dma_start(out=b2b[:], in_=b2s[:])
            zpad = cpool.tile([128, BW - 2048], BF16, tag="zpad")
            nc.vector.memset(zpad[:], NEG)

            # FF prefetch destinations (loads issued mid-attention)
            rows1_t = cpool.tile([128, D], BF16, tag="rows1")
            wres_t = cpool.tile([128, 2 * D], BF16, tag="wres2")
            rsx_t = cpool.tile([128, 2 * D], BF16, tag="rsx")

            # w1/w2 as 4 big block tiles each, one batched DMA per block
            # (3-dim src AP, k-chunks side by side in the free dim)
            w1big = [None] * 4
            w2big = [None] * 4

            # weight-block loads are CHUNKED (4 x 512KB) so they interleave
            # with latency-critical small transfers on the shared DMA pool.
            # During scores they ride the Act queue (no WAR -> never park);
            # FF-time reloads use SP (their WAR parks are harmless there).
            def load_w1_block(blk, eng=None):
                eng = eng or nc.sync
                bt = w1pool.tile([128, 8192], BF16, tag="w1big",
                                 bufs=3, name=f"w1big{blk}")
                for c in range(4):
                    src = bass_rust.AP(
                        tensor=w1T.ap().tensor,
                        offset=1024 * (blk % 4) + 2 * c * 128 * DI,
                        ap=[[DI, 128], [128 * DI, 2], [1, 1024]])
                    eng.dma_start(out=bt[:, 2048 * c:2048 * c + 2048],
                                  in_=src)
                w1big[blk % 4] = bt

            def load_w2_block(blk, eng=None):
                eng = eng or nc.sync
                bt = w2pool.tile([128, 8192], BF16, tag="w2big",
                                 bufs=2, name=f"w2big{blk}")
                for c in range(4):
                    src = bass_rust.AP(
                        tensor=w2T.ap().tensor,
                        offset=(8 * (blk % 4) + 2 * c) * 128 * D,
                        ap=[[D, 128], [128 * D, 2], [1, 1024]])
                    eng.dma_start(out=bt[:, 2048 * c:2048 * c + 2048],
                                  in_=src)
                w2big[blk % 4] = bt

            # shared attention tensors (outlive the P1 scope)
            woT_t = []
            for k in range(HD_G // 128):
                tt = cpool.tile([128, D], BF16, tag=f"woT{k}", name=f"woT{k}")
                woT_t.append(tt)
            k_fm, rk_fm, qh_fm = [], [], []
            for m in range(2):
                k_fm.append(cpool.tile([128, KLEN], BF16, tag=f"kfm{m}",
                                       name=f"kfm{m}"))
                rk_fm.append(cpool.tile([128, KLEN], BF16, tag=f"rkfm{m}",
                                        name=f"rkfm{m}"))
                qh_fm.append(cpool.tile([128, QLEN], BF16, tag=f"qhfm{m}",
                                        name=f"qhfm{m}"))
            v_tok = []
            for kt in range(NKT):
                v_tok.append(cpool.tile([128, 65 * HPG], BF16, tag=f"vtok{kt}",
                                        name=f"vtok{kt}"))
            qt_fm = []
            for m in range(2):
                qt_fm.append(cpool.tile([128, QLEN], BF16, tag=f"qtfm{m}",
                                        name=f"qtfm{m}"))
            vecT_fm = {}
            for m in range(2):
                for hf in range(2):
                    vecT_fm[(m, hf)] = cpool.tile(
                        [128, QLEN // 2], BF16, tag=f"vecT{m}_{hf}",
                        name=f"vecT{m}_{hf}")

            # ================ P1 scope (cat/pw/qt/r die with it) ========
            with tc.tile_pool(name="p1", bufs=1) as apool:

                # ---------------- P1: inputs then projection weights ------
                # cat/pw/r as big sliced tiles; loads stream in consumption
                # order as a few batched 3-dim-AP DMAs on the SP queue
                catb = apool.tile([128, NDC * KLEN], BF16, tag="catb",
                                  name="catb")
                pwb = apool.tile([128, NDC * 4 * HD_G], BF16, tag="pwb",
                                 name="pwb")
                rb = apool.tile([128, NDC * KLEN], BF16, tag="rb", name="rb")
                cat_t = [catb[:, KLEN * k:KLEN * k + KLEN] for k in range(NDC)]
                pw = [pwb[:, 1024 * k:1024 * k + 1024] for k in range(NDC)]
                r_t = [rb[:, KLEN * k:KLEN * k + KLEN] for k in range(NDC)]

                def cat_blk(c0, w):
                    dst = bass_rust.AP(
                        tensor=catb[:].tensor, offset=c0,
                        ap=[[NDC * KLEN, 128], [KLEN, NDC], [1, w]])
                    src = bass_rust.AP(
                        tensor=cat_fm.ap().tensor, offset=c0,
                        ap=[[KLEN, 128], [128 * KLEN, NDC], [1, w]])
                    nc.sync.dma_start(out=dst, in_=src)

                def pw_part(c0, w):
                    dst = bass_rust.AP(
                        tensor=pwb[:].tensor, offset=c0,
                        ap=[[NDC * 1024, 128], [1024, NDC], [1, w]])
                    src = bass_rust.AP(
                        tensor=wpk.ap().tensor, offset=c0,
                        ap=[[1024, 128], [128 * 1024, NDC], [1, w]])
                    nc.sync.dma_start(out=dst, in_=src)

                pw_part(3 * HD_G, HD_G)      # wv first (V proj gate)
                cat_blk(0, 256)              # V kt0-1 gate: small first chunk
                cat_blk(256, 256)
                pw_part(0, 2 * HD_G)         # wk | wq (K starts early)
                cat_blk(512, 512)
                cat_blk(1024, 512)
                cat_blk(1536, 512)
                pw_part(2 * HD_G, HD_G)      # wr
                rsrc = bass_rust.AP(
                    tensor=r_fm.ap().tensor, offset=0,
                    ap=[[KLEN, 128], [128 * KLEN, NDC], [1, KLEN]])
                nc.sync.dma_start(out=rb[:], in_=rsrc)
                for k in range(HD_G // 128):
                    nc.sync.dma_start(out=woT_t[k][:],
                                      in_=woT[128 * k:128 * k + 128, :])

                def pw_sl(k, which, m):
                    base = {"wkT": 0, "wqT": 1, "wrT": 2, "wvT": 3}[which] * HD_G
                    return pw[k][:, base + 128 * m:base + 128 * m + 128]

                # V proj (token-major, interleaved ones cols) and K proj,
                # interleaved so each only needs the cat column blocks that
                # have landed (V kt-tiles pace with cat_blk arrivals, K rides
                # in between)
                def v_tile(kt):
                    vt = v_tok[kt]
                    ps = psB.tile([128, HD_G], F32, tag="psB", name="psv")
                    for k in range(NDC):
                        nc.tensor.matmul(
                            ps[:], cat_t[k][:, 128 * kt:128 * kt + 128],
                            pw[k][:, 3 * HD_G:4 * HD_G],
                            start=(k == 0), stop=(k == NDC - 1))
                    for hh in range(HPG):
                        nc.scalar.activation(vt[:, 65 * hh:65 * hh + 64],
                                             ps[:, 64 * hh:64 * hh + 64],
                                             AF.Copy)
                        nc.vector.memset(vt[:, 65 * hh + 64:65 * hh + 65], 1.0)

                def k_tile(m, n):
                    ps = psA.tile([128, 512], F32, tag="psA", name="psk")
                    for k in range(NDC):
                        nc.tensor.matmul(
                            ps[:], pw_sl(k, "wkT", m),
                            cat_t[k][:, 512 * n:512 * n + 512],
                            start=(k == 0), stop=(k == NDC - 1))
                    nc.vector.tensor_copy(
                        k_fm[m][:, 512 * n:512 * n + 512], ps[:])

                for kt in range(4):
                    v_tile(kt)
                k_tile(0, 0)
                k_tile(1, 0)
                for kt in range(4, 8):
                    v_tile(kt)
                k_tile(0, 1)
                k_tile(1, 1)
                for kt in range(8, 12):
                    v_tile(kt)
                k_tile(0, 2)
                k_tile(1, 2)
                for kt in range(12, 16):
                    v_tile(kt)
                k_tile(0, 3)
                k_tile(1, 3)
                # Q proj (cat cols MLEN.. only; last cat consumer)
                for m in range(2):
                    for n in range(QLEN // 512):
                        ps = psA.tile([128, 512], F32, tag="psA", name="psq")
                        for k in range(NDC):
                            nc.tensor.matmul(
                                ps[:], pw_sl(k, "wqT", m),
                                cat_t[k][:, MLEN + 512 * n:MLEN + 512 * n + 512],
                                start=(k == 0), stop=(k == NDC - 1))
                        # (q + bias) * SCALE on DVE, cast to bf16
                        nc.vector.tensor_scalar(
                            out=qh_fm[m][:, 512 * n:512 * n + 512], in0=ps[:],
                            scalar1=bias_t[:, m:m + 1], scalar2=SCALE,
                            op0=ALU.add, op1=ALU.mult)
                        nc.scalar.activation(
                            qt_fm[m][:, 512 * n:512 * n + 512], ps[:],
                            AF.Identity, scale=SCALE,
                            bias=bias_t[:, 6 + m:7 + m])
                # NEG pad columns of B (only needed before first oblique read)
                zsrc = bass_rust.AP(
                    tensor=zpad[:].tensor, offset=zpad[:].offset,
                    ap=[[BW - 2048, 128], [0, NQT], [1, BW - 2048]])
                for h in range(HPG):
                    nc.scalar.dma_start(out=bpad(h), in_=zsrc)
                for m in range(2):
                    for n in range(KLEN // 512):
                        ps = psA.tile([128, 512], F32, tag="psA", name="psr")
                        for k in range(NDC):
                            nc.tensor.matmul(
                                ps[:], pw_sl(k, "wrT", m),
                                r_t[k][:, 512 * n:512 * n + 512],
                                start=(k == 0), stop=(k == NDC - 1))
                        nc.vector.tensor_copy(
                            rk_fm[m][:, 512 * n:512 * n + 512], ps[:])

                # ---------------- P2: attention per head ----------------
                def head_slices(h):
                    m, p0 = h // 2, 64 * (h % 2)
                    return (m, p0, qh_fm[m][p0:p0 + 64, :],
                            qt_fm[m][p0:p0 + 64, :],
                            k_fm[m][p0:p0 + 64, :],
                            rk_fm[m][p0:p0 + 64, :])

                def b_phase(h):
                    m, p0, qh_h, qt_h, k_h, rk_h = head_slices(h)
                    # B = q_tilde @ rk^T -> f32 PSUM -> SBUF bf16 -> one DMA
                    # per q-tile; for qt<=3 the leading 512 cols are never
                    # read back -> skip them. Writes go on the Activation
                    # queue so oblique reads aren't queued behind them.
                    for qt in range(NQT):
                        # exact causal band: rows of q-tile qt only ever read
                        # back columns c >= 896 - 128*qt
                        c0 = max(0, 896 - 128 * qt)
                        bs = wpool.tile([128, KLEN], BF16, tag="big",
                                        bufs=2, name=f"bs{h}_{qt}")
                        # staging alternates DVE/Act per chunk
                        for ct in range(c0 // 512, KLEN // 512):
                            lo = max(c0, 512 * ct)
                            wdt = 512 * (ct + 1) - lo
                            pb = psBB.tile([128, 512], F32, tag="psBB",
                                           name="psb")
                            nc.tensor.matmul(
                                pb[:, 0:wdt], qt_h[:, 128 * qt:128 * qt + 128],
                                rk_h[:, lo:lo + wdt],
                                start=True, stop=True)
                            sl = bs[:, lo:lo + wdt]
                            if (h + qt + ct) % 2 == 0:
                                nc.vector.tensor_copy(sl, pb[:, 0:wdt])
                            else:
                                nc.scalar.activation(sl, pb[:, 0:wdt], AF.Copy)
                        nc.sync.dma_start(
                            out=bwrite(h, qt, c0, KLEN - c0),
                            in_=bs[:, c0:KLEN])

                def scores(h, qh2):
                    # key-major scores: S^T[key, q] = AC^T + BD^T; BD comes
                    # back from DRAM per-ktb [128, 4x512] bf16, cast to f32
                    # on DVE/Pool (SBUF->SBUF; Pool cannot touch PSUM), then
                    # PE-transpose-accumulated into the AC^T psum group
                    m, p0, qh_h, qt_h, k_h, rk_h = head_slices(h)
                    probT = [wpool.tile([128, 512], BF16, tag=f"pT{kt}",
                                        bufs=1, name=f"pT{kt}_{h}_{qh2}")
                             for kt in range(NKT)]
                    nktb = 3 if qh2 == 0 else 4
                    bd_t = {}
                    for ktb in range(nktb):
                        bd16 = wpool.tile([128, 2048], BF16, tag="bd16",
                                          bufs=4, name=f"bd16_{ktb}")
                        # oblique reads on SP: by scores time the prefetch
                        # stream has drained, so their WAR parks self-pace
                        nc.sync.dma_start(out=bd16[:], in_=obl(h, qh2, ktb))
                        for qi, qt in enumerate(range(4 * qh2, 4 * qh2 + 4)):
                            kmax = min(qt + 8, NKT - 1)
                            wdt = min(512, (kmax + 1 - 4 * ktb) * 128)
                            if wdt <= 0:
                                continue
                            bd = wpool.tile([128, 512], F32, tag="bd",
                                            bufs=5, name=f"bd{qt}_{ktb}")
                            # cast engines: first half can use Pool (its
                            # queue is clear until collective(0) is reached);
                            # second half must avoid Pool (collective parks
                            # its SEQ) so Act takes the overflow
                            if qh2 == 0 and (qi + ktb) % 2 == 0:
                                eng = nc.gpsimd
                            else:
                                eng = nc.vector
                            if eng is nc.scalar:
                                eng.activation(bd[:, 0:wdt],
                                               bd16[:, 512 * qi:512 * qi + wdt],
                                               AF.Copy)
                            else:
                                eng.tensor_copy(bd[:, 0:wdt],
                                                bd16[:, 512 * qi:512 * qi + wdt])
                            bd_t[(qt, ktb)] = bd
                    for kt in range(4 * nktb):
                        qts = [qt for qt in range(4 * qh2, 4 * qh2 + 4)
                               if qt >= kt - 8]
                        if not qts:
                            continue
                        if qh2 == 1:
                            pspool, pstag = [(psA, "psA"), (psB, "psB"),
                                             (psBB, "psBB")][kt % 3]
                        else:
                            pspool, pstag = ((psA, "psA") if kt % 2 == 0
                                             else (psB, "psB"))
                        ps = pspool.tile([128, 512], F32, tag=pstag,
                                         name="pss")
                        sub0 = 128 * (qts[0] - 4 * qh2)
                        w0 = 128 * (qts[-1] + 1 - qts[0])
                        nc.tensor.matmul(
                            ps[:, sub0:sub0 + w0],
                            k_h[:, 128 * kt:128 * kt + 128],
                            qh_h[:, 512 * qh2 + sub0:512 * qh2 + sub0 + w0],
                            start=True, stop=False)
                        for i, qt in enumerate(qts):
                            bd = bd_t[(qt, kt // 4)]
                            bo = 128 * (kt % 4)
                            sub = 128 * (qt - 4 * qh2)
                            nc.tensor.matmul(ps[:, sub:sub + 128],
                                             bd[:, bo:bo + 128],
                                             ident[:], is_transpose=True,
                                             start=False,
                                             stop=(i == len(qts) - 1))
                        blo = qts[0]
                        sub = 128 * (blo - 4 * qh2)
                        w = 128 * (4 * qh2 + 4 - blo)
                        nc.scalar.activation(probT[kt][:, sub:sub + w],
                                             ps[:, sub:sub + w], AF.Exp)
                    for qt in range(4 * qh2, 4 * qh2 + 4):
                        vec_qt(h, qh2, qt, probT)

                def vec_qt(h, qh2, qt, probT):
                    m, p0 = h // 2, 64 * (h % 2)
                    kmax = min(qt + 8, NKT - 1)
                    pv = psVT.tile([128, 65], F32, tag="psVT", name="pv")
                    sub = 128 * (qt - 4 * qh2)
                    for kt in range(kmax + 1):
                        nc.tensor.matmul(
                            pv[:], probT[kt][:, sub:sub + 128],
                            v_tok[kt][:, 65 * h:65 * h + 65],
                            start=(kt == 0), stop=(kt == kmax))
                    rec = wpool.tile([128, 1], F32, tag="rec", name="rec")
                    nc.vector.reciprocal(rec[:], pv[:, 64:65])
                    vn = wpool.tile([128, 64], BF16, tag="vn", name="vn")
                    nc.vector.tensor_scalar_mul(vn[:], pv[:, 0:64], rec[:])
                    pt = psVT.tile([64, 128], BF16, tag="psVT", name="ptr")
                    nc.tensor.matmul(pt[:], vn[:], identb[:],
                                     is_transpose=True,
                                     start=True, stop=True)
                    dst = vecT_fm[(m, qh2)][p0:p0 + 64,
                                            128 * (qt % 4):128 * (qt % 4) + 128]
                    if qh2 == 1:
                        nc.vector.tensor_copy(dst, pt[:])
                    else:
                        nc.scalar.activation(dst, pt[:], AF.Copy)

                def p3_half(qh2):
                    # partial attn_out for this half's 4 q-tiles, then the
                    # quad ReduceScatter for it and the rs_out readback
                    for qg in range(2 * qh2, 2 * qh2 + 2):
                        ao = wpool.tile([128, KLEN], BF16, tag="big",
                                        bufs=2, name=f"ao{qg}")
                        for qi in range(2):
                            qt = 2 * qg + qi
                            for n in range(D // 512):
                                ps = psA.tile([128, 512], F32, tag="psA",
                                              name="pso")
                                for k in range(2):
                                    nc.tensor.matmul(
                                        ps[:],
                                        vecT_fm[(k, qt // 4)][:, 128 * (qt % 4):
                                                              128 * (qt % 4) + 128],
                                        woT_t[k][:, 512 * n:512 * n + 512],
                                        start=(k == 0), stop=(k == 1))
                                sl = ao[:, D * qi + 512 * n:
                                        D * qi + 512 * n + 512]
                                # DVE both halves: it is idle by p3 time,
                                # and the staging must not queue behind the
                                # Act exp backlog (it gates the collective)
                                nc.vector.tensor_copy(sl, ps[:])
                        # both q-tiles in one Act-queue write (staging Act
                        # copy precedes it in-queue; SP stream untouched)
                        dst = bass_rust.AP(
                            tensor=attn_part.ap().tensor, offset=256 * qg * D,
                            ap=[[D, 128], [128 * D, 2], [1, D]])
                        nc.sync.dma_start(out=dst, in_=ao[:, 0:2 * D])
                    s = qh2
                    if single_sim:
                        nc.sync.dma_start(
                            out=rs_out[128 * s:128 * s + 128, :],
                            in_=attn_part[512 * s:512 * s + 128, :])
                    else:
                        nc.gpsimd.collective_compute(
                            "ReduceScatter", ALU.add,
                            replica_groups=[[0, 1, 2, 3], [4, 5, 6, 7]],
                            ins=[attn_part[512 * s:512 * s + 512, :]],
                            outs=[rs_out[128 * s:128 * s + 128, :]])
                    if s == 1:
                        # readback(1) on the Pool SWDGE queue right behind
                        # its collective (SP would park ahead of the FF
                        # sweep's weight reloads)
                        nc.gpsimd.dma_start(
                            out=rsx_t[:, D:2 * D],
                            in_=rs_out[128:256, :])

                g1r = rows1_t[:, 0:D]

                def ln_normalize(x_t, z_out, nm):
                    # DVE-only LN via bn_stats/bn_aggr (two 512-halves), then
                    # z = rstd*x - m*rstd via one tensor_scalar pass
                    st = wpool.tile([128, 12], F32, tag="lnst", name=f"lnst{nm}")
                    nc.vector.bn_stats(st[:, 0:6], x_t[:, 0:512])
                    nc.vector.bn_stats(st[:, 6:12], x_t[:, 512:1024])
                    mv = wpool.tile([128, 2], F32, tag="lnmv", name=f"lnmv{nm}")
                    nc.vector.bn_aggr(mv[:], st[:])
                    ve = wpool.tile([128, 1], F32, tag="lnve", name=f"lnve{nm}")
                    nc.vector.tensor_scalar(out=ve[:], in0=mv[:, 1:2],
                                            scalar1=1e-5, scalar2=0.0,
                                            op0=ALU.add, op1=ALU.add)
                    rc = wpool.tile([128, 1], F32, tag="lnrc", name=f"lnrc{nm}")
                    nc.vector.reciprocal(rc[:], ve[:])
                    rstd = wpool.tile([128, 1], F32, tag="lnrstd",
                                      name=f"lnrstd{nm}")
                    nc.scalar.activation(rstd[:], rc[:], AF.Sqrt)
                    nb = wpool.tile([128, 1], F32, tag="lnnb", name=f"lnnb{nm}")
                    nc.vector.tensor_tensor(out=nb[:], in0=mv[:, 0:1],
                                            in1=rstd[:], op=ALU.mult)
                    nc.vector.tensor_scalar_mul(nb[:], nb[:], -1.0)
                    nc.vector.tensor_scalar(out=z_out, in0=x_t[:],
                                            scalar1=rstd[:], scalar2=nb[:],
                                            op0=ALU.mult, op1=ALU.add)

                def ln1_tile(t):
                    nm = f"l1{t}"
                    xt = wpool.tile([128, D], F32, tag="xres", bufs=2,
                                    name=f"xres{t}")
                    st = wpool.tile([128, 12], F32, tag="lnst", name=f"lnst{nm}")
                    for hf in range(2):
                        sl = slice(512 * hf, 512 * hf + 512)
                        nc.vector.tensor_tensor(
                            out=xt[:, sl],
                            in0=wres_t[:, D * t + 512 * hf:D * t + 512 * hf + 512],
                            in1=rsx_t[:, D * t + 512 * hf:D * t + 512 * hf + 512],
                            op=ALU.add)
                        nc.vector.bn_stats(st[:, 6 * hf:6 * hf + 6], xt[:, sl])
                    mv = wpool.tile([128, 2], F32, tag="lnmv", name=f"lnmv{nm}")
                    nc.vector.bn_aggr(mv[:], st[:])
                    ve = wpool.tile([128, 1], F32, tag="lnve", name=f"lnve{nm}")
                    nc.vector.tensor_scalar(out=ve[:], in0=mv[:, 1:2],
                                            scalar1=1e-5, scalar2=0.0,
                                            op0=ALU.add, op1=ALU.add)
                    rc = wpool.tile([128, 1], F32, tag="lnrc", name=f"lnrc{nm}")
                    nc.vector.reciprocal(rc[:], ve[:])
                    rstd = wpool.tile([128, 1], F32, tag="lnrstd",
                                      name=f"lnrstd{nm}")
                    nc.scalar.activation(rstd[:], rc[:], AF.Sqrt)
                    nb = wpool.tile([128, 1], F32, tag="lnnb", name=f"lnnb{nm}")
                    nc.vector.tensor_tensor(out=nb[:], in0=mv[:, 0:1],
                                            in1=rstd[:], op=ALU.mult)
                    nc.vector.tensor_scalar_mul(nb[:], nb[:], -1.0)
                    for hf in range(2):
                        sl = slice(512 * hf, 512 * hf + 512)
                        nc.vector.tensor_scalar(out=zt[t][:, sl], in0=xt[:, sl],
                                                scalar1=rstd[:], scalar2=nb[:],
                                                op0=ALU.mult, op1=ALU.add)

                def zT_zg(t):
                    # transpose z to feature-major + z*g1 residual staging
                    for k in range(NDC):
                        pt = psBB.tile([128, 128], BF16, tag="psBB",
                                       name=f"ptf{t}_{k}")
                        nc.tensor.matmul(pt[:], zt[t][:, 128 * k:128 * k + 128],
                                         identb[:], is_transpose=True,
                                         start=True, stop=True)
                        nc.scalar.activation(out1_fm[k][:, 128 * t:128 * t + 128],
                                             pt[:], AF.Copy)
                    nc.vector.tensor_tensor(out=zgs[t][:], in0=zt[t][:], in1=g1r,
                                            op=ALU.mult)

                # B per head pipelined ahead of its scores pair (probT and
                # all scores inputs live outside this scope, so scores may
                # interleave here; p1 closes after the last b_phase)
                b_phase(0)
                b_phase(1)
                scores(0, 0)
                b_phase(2)
                scores(1, 0)
                b_phase(3)

            # ====== FF-weight pools open once the P1 SBUF is recycled ====
            _es = ExitStack()
            w1pool = _es.enter_context(tc.tile_pool(name="w1p", bufs=1))
            w2pool = _es.enter_context(tc.tile_pool(name="w2p", bufs=1))
            # prefetch all FF weights on the Act queue: no deps, no parks,
            # and the SP stream (B writes + obliques) keeps self-pacing
            nc.scalar.dma_start(out=rows1_t[:], in_=rows4[:, 0:D])
            nc.scalar.dma_start(out=wres_t[:], in_=wres2[:])
            scores(2, 0)
            scores(3, 0)
            p3_half(0)
            # block-0 loads trigger here: the first-half window is DMA-pool
            # saturated (B writes + obliques); this one has plenty of slack
            load_w1_block(0, nc.scalar)
            load_w2_block(0, nc.scalar)
            scores(0, 1)
            scores(1, 1)
            scores(2, 1)
            scores(3, 1)
            # block-1 loads here: out of the DMA-saturated scores window,
            # still well ahead of FF sweep-0 mi=8
            load_w1_block(1, nc.scalar)
            load_w2_block(1, nc.scalar)
            # readback(0) on SP: every oblique is already queued, so its
            # park (until RS(0) lands) blocks nothing time-critical
            nc.sync.dma_start(out=rsx_t[:, 0:D], in_=rs_out[0:128, :])
            p3_half(1)           # issues RS(1); staging is Act-only
            ln1_tile(0)          # after p3: its Act Sqrt can't delay the
                                 # attn writes that gate RS(1)

            # ================ FF scope ================
            # t-split sweeps: all of t=0's FF1+FF2 first (independent of
            # RS(1)), so the second ReduceScatter hides under it; LN2(t0)
            # hides under the t=1 sweep; only LN2(t1) is an exposed tail.
            with tc.tile_pool(name="ff", bufs=1) as fpool:
                # FF staging piggybacks on wpool tags that are dead once
                # scores complete (probT / bd16), instead of new SBUF
                for k in range(NDC):
                    out1_fm[k] = wpool.tile([128, ROWS], BF16, tag=f"pT{k}",
                                            bufs=1, name=f"o1fm{k}")
                for t in range(ROWS // 128):
                    zgs[t] = wpool.tile([128, D], BF16, tag="bd16",
                                        bufs=4, name=f"zg{t}")
                rows23_t = wpool.tile([128, 2 * D], BF16, tag="bd16",
                                      bufs=4, name="rows23")
                nc.sync.dma_start(out=rows23_t[:], in_=rows4[:, D:3 * D])
                g2r = rows23_t[:, 0:D]
                lb2r = rows23_t[:, D:2 * D]

                hps = {}
                hps[(0, 0)] = psB.tile([128, 512], F32, tag="psB", name="h2ps00")
                hps[(0, 1)] = psB.tile([128, 512], F32, tag="psB", name="h2ps01")
                hps[(1, 0)] = psVT.tile([128, 512], F32, tag="psVT", name="h2ps10")
                hps[(1, 1)] = psVT.tile([128, 512], F32, tag="psVT", name="h2ps11")
                # inject b2 + ln1_b into each h2 group BEFORE the RS-gated
                # zT(0): out[p, j] += 1 * b2[j]
                for (t, n), hp in hps.items():
                    nc.tensor.matmul(hp[:], onesr[:, 0:128],
                                     b2b[:, 512 * n:512 * n + 512],
                                     start=True, stop=False)
                zT_zg(0)

                def ff1_mi(mi, t):
                    blk, mo = mi // 8, 128 * (mi % 8)
                    pool_, tag_ = (psA, "psA") if mi % 2 == 0 else (psBB, "psBB")
                    ps = pool_.tile([128, 128], F32, tag=tag_,
                                    name=f"psh1_{mi}_{t}")
                    for k in range(NDC):
                        nc.tensor.matmul(
                            ps[:], w1big[blk][:, 1024 * k + mo:1024 * k + mo + 128],
                            out1_fm[k][:, 128 * t:128 * t + 128],
                            start=(k == 0), stop=(k == NDC - 1))
                    ht = fpool.tile([128, 128], BF16, tag="h1T", bufs=3,
                                    name=f"h1T{mi}_{t}")
                    nc.scalar.activation(ht[:], ps[:], AF.Relu,
                                         bias=b1c_t[:, mi:mi + 1])
                    return ht

                def ff2_mi(mi, t, ht, last):
                    blk = mi // 8
                    for n in range(D // 512):
                        nc.tensor.matmul(
                            hps[(t, n)][:], ht[:],
                            w2big[blk][:, 1024 * (mi % 8) + 512 * n:
                                        1024 * (mi % 8) + 512 * n + 512],
                            start=False, stop=last)

                def ln2_tile(t):
                    # x2 = z*g1 + core + b2 + ln1_b lives ENTIRELY in the
                    # FF2 psum (zg injected by an identity matmul), so LN2
                    # reads stats and z straight from psum; y goes out in
                    # column halves so the first DMA launches early
                    nm = f"l2{t}"
                    st = wpool.tile([128, 12], F32, tag="lnst", name=f"lnst{nm}")
                    for n in range(D // 512):
                        nc.vector.bn_stats(st[:, 6 * n:6 * n + 6],
                                           hps[(t, n)][:])
                    mv = wpool.tile([128, 2], F32, tag="lnmv", name=f"lnmv{nm}")
                    nc.vector.bn_aggr(mv[:], st[:])
                    ve = wpool.tile([128, 1], F32, tag="lnve", name=f"lnve{nm}")
                    nc.vector.tensor_scalar(out=ve[:], in0=mv[:, 1:2],
                                            scalar1=1e-5, scalar2=0.0,
                                            op0=ALU.add, op1=ALU.add)
                    rc = wpool.tile([128, 1], F32, tag="lnrc", name=f"lnrc{nm}")
                    nc.vector.reciprocal(rc[:], ve[:])
                    rstd = wpool.tile([128, 1], F32, tag="lnrstd",
                                      name=f"lnrstd{nm}")
                    nc.scalar.activation(rstd[:], rc[:], AF.Sqrt)
                    nb = wpool.tile([128, 1], F32, tag="lnnb", name=f"lnnb{nm}")
                    nc.vector.tensor_tensor(out=nb[:], in0=mv[:, 0:1],
                                            in1=rstd[:], op=ALU.mult)
                    nc.vector.tensor_scalar_mul(nb[:], nb[:], -1.0)
                    z2 = wpool.tile([128, D], BF16, tag="bd", bufs=5,
                                    name=f"z2_{t}")
                    yt = wpool.tile([128, D], F32, tag="big", bufs=2,
                                    name=f"y_{t}")
                    for n in range(D // 512):
                        sl = slice(512 * n, 512 * n + 512)
                        nc.vector.tensor_scalar(out=z2[:, sl],
                                                in0=hps[(t, n)][:],
                                                scalar1=rstd[:], scalar2=nb[:],
                                                op0=ALU.mult, op1=ALU.add)
                        nc.vector.tensor_tensor(out=yt[:, sl], in0=z2[:, sl],
                                                in1=g2r[:, sl], op=ALU.mult)
                        nc.vector.tensor_tensor(out=yt[:, sl], in0=yt[:, sl],
                                                in1=lb2r[:, sl], op=ALU.add)
                        nc.sync.dma_start(
                            out=y[128 * t:128 * t + 128, 512 * n:512 * n + 512],
                            in_=yt[:, sl])

                # t=0 sweep walks w1/w2 blocks 0..3; the t=1 sweep runs in
                # REVERSE mi order so blocks 3,2 are still resident at the
                # turn and only 1,0 reload (hidden under FF compute).
                # FF2 trails FF1 by one mi so the relu latency never stalls
                # the PE queue.
                pend = None
                # zg(0) joins the hps(0,*) groups up front (group order is
                # free), keeping the groups' stop on the last FF2 matmul
                for n in range(D // 512):
                    nc.tensor.matmul(hps[(0, n)][:], identb[:],
                                     zgs[0][:, 512 * n:512 * n + 512],
                                     start=False, stop=False)
                for mi in range(NMI):
                    if mi == 0:
                        load_w1_block(2)
                        load_w2_block(2)
                    elif mi == 8:
                        load_w1_block(3)
                        load_w2_block(3)
                    elif mi == 24:
                        ln1_tile(1)      # DVE-only chain; parks idle DVE
                    ht = ff1_mi(mi, 0)
                    if pend is not None:
                        ff2_mi(*pend, last=False)
                    pend = (mi, 0, ht)
                ff2_mi(*pend, last=True)
                pend = None
                zT_zg(1)
                for n in range(D // 512):
                    nc.tensor.matmul(hps[(1, n)][:], identb[:],
                                     zgs[1][:, 512 * n:512 * n + 512],
                                     start=False, stop=False)
                for i, mi in enumerate(reversed(range(NMI))):
                    if i == 0:
                        # w1 blk1 is STILL RESIDENT (bufs=3); only w2 reloads
                        load_w2_block(5)
                    elif i == 8:
                        # advance w1 rotation past blk1's live slot, then
                        # reload blk0 into blk2's slot (free after i=15)
                        w1pool.tile([128, 8192], BF16, tag="w1big",
                                    bufs=3, name="w1skip")
                        load_w1_block(4)
                        load_w2_block(4)
                    elif i == 2:
                        ln2_tile(0)      # hps(0,·) complete; DVE/Pool only
                    ht = ff1_mi(mi, 1)
                    if pend is not None:
                        ff2_mi(*pend, last=False)
                    pend = (mi, 1, ht)
                ff2_mi(*pend, last=True)
                ln2_tile(1)
            _es.close()

    nc.compile()
    return nc


def _prep_inputs(w, r, mems, W_qkv, W_r, W_o, r_w_bias, r_r_bias,
                 ln1_g, ln1_b, ff_W1, ff_b1, ff_W2, ff_b2, ln2_g, ln2_b,
                 attn_mask=None):
    import ml_dtypes
    f32 = np.float32
    bf16 = ml_dtypes.bfloat16
    cat = np.concatenate([mems, w], axis=0)            # [KLEN, B, D]
    cat_fm = [np.ascontiguousarray(cat[:, b, :].T).astype(bf16)
              for b in range(BSZ)]
    r_fm = np.ascontiguousarray(r.T).astype(bf16)
    # LN1 affine folded into FF weights
    W1p = np.asarray(ff_W1, f32) * np.asarray(ln1_g, f32)[None, :]
    b1p = np.asarray(ff_b1, f32) + np.asarray(ff_W1, f32) @ np.asarray(ln1_b, f32)
    b2p = np.asarray(ff_b2, f32) + np.asarray(ln1_b, f32)
    w1T = np.ascontiguousarray(W1p.T).astype(bf16)     # [D, DI]
    w2T = np.ascontiguousarray(np.asarray(ff_W2, f32).T).astype(bf16)
    woT_full = np.ascontiguousarray(W_o.T, dtype=f32)  # [H*DH, D]
    b1c = np.ascontiguousarray(b1p.reshape(NMI, 128).T)  # [128, NMI]
    rowb = lambda v: np.broadcast_to(
        np.asarray(v, f32).reshape(1, D), (128, D))
    rows4 = np.ascontiguousarray(np.concatenate(
        [rowb(ln1_g), rowb(ln2_g), rowb(ln2_b)], axis=1)).astype(bf16)

    in_maps = []
    for c in range(N_CORES):
        b, g = c // 4, c % 4
        sl = slice(HD_G * g, HD_G * g + HD_G)
        wkT = np.asarray(W_qkv, f32)[H * DH:2 * H * DH][sl].T
        wqT = np.asarray(W_qkv, f32)[0:H * DH][sl].T
        wrT = np.asarray(W_r, f32)[sl].T
        wvT = np.asarray(W_qkv, f32)[2 * H * DH:3 * H * DH][sl].T
        wpk = np.concatenate([wkT, wqT, wrT, wvT], axis=1)  # [D, 4*HD_G]
        rwbv = np.asarray(r_w_bias, f32).reshape(-1)[sl]
        rrbv = np.asarray(r_r_bias, f32).reshape(-1)[sl]
        bias = np.stack([
            rwbv[0:128], rwbv[128:256], rrbv[0:128], rrbv[128:256],
            rwbv[0:128] * SCALE, rwbv[128:256] * SCALE,
            rrbv[0:128] * SCALE, rrbv[128:256] * SCALE,
        ], axis=1)                                          # [128, 8]
        wres2 = np.concatenate(
            [np.asarray(w, f32)[128 * g:128 * g + 128, b, :],
             np.asarray(w, f32)[512 + 128 * g:512 + 128 * g + 128, b, :]],
            axis=1)                                         # [128, 2*D]
        m = {
            "cat_fm": cat_fm[b],
            "r_fm": r_fm,
            "wpk": np.ascontiguousarray(wpk).astype(bf16),
            "biases": np.ascontiguousarray(bias),
            "woT": np.ascontiguousarray(woT_full[sl]).astype(bf16),
            "w1T": w1T, "b1c": b1c, "w2T": w2T,
            "b2s": np.ascontiguousarray(b2p.reshape(1, D)).astype(bf16),
            "rows4": rows4,
            "wres2": np.ascontiguousarray(wres2).astype(bf16),
        }
        in_maps.append(m)
    return in_maps


def kernel(**inputs):
    from concourse.bass_utils import run_bass_kernel_spmd
    nc = _build()
    in_maps = _prep_inputs(**{k: np.asarray(v) for k, v in inputs.items()})
    res = run_bass_kernel_spmd(nc, in_maps, list(range(N_CORES)))
    out = np.empty((QLEN, BSZ, D), np.float32)
    for c in range(N_CORES):
        b, g = c // 4, c % 4
        yv = res.results[c]["y"]
        out[128 * g:128 * g + 128, b, :] = yv[0:128]
        out[512 + 128 * g:512 + 128 * g + 128, b, :] = yv[128:256]
    return out



# revision 135
# speedup vs baseline: 1.0186x; 1.0186x over previous
"""Trainium2 Bass kernel for a Transformer-XL (MemTransformerLM) layer.

Sharding over 8 cores: core c = (b = c//4, head-group g = c%4 of 4 heads).
Each core computes its 4 heads' attention for its batch, a partial
attn_out = vec @ W_o[:, heads].T, then a ReduceScatter(+) over the quad
[[0..3],[4..7]] scatters query rows -> each core does LN1+FF+LN2 on its
256 rows. Host reassembles [1024, 2, 1024].

rel_shift: B = q_tilde @ rk^T goes to f32 PSUM, staged to SBUF bf16
(band-trimmed: q-tile qt only writes columns >= 896-128*qt), written per
q-tile to DRAM with row stride 2176; BD[i,j] = B[i, j-i+1023] is read
back with batched oblique APs (one [128, 4x512] read per 512-key block
covering 4 q-tiles), cast to f32 on DVE/Pool, and PE-transpose-
accumulated (f32, 2 cyc/row) into the AC^T PSUM group, so S^T = AC^T +
BD^T lands in PSUM with no extra vector pass. Mask: pad columns
[2048, 2176) are -1e30 so the band tile kt == qt+8 picks up NEG in its
upper triangle; tiles kt > qt+8 are skipped. Softmax skips the
max-subtraction; the denominator comes free from a ones-column in v.

LN1's affine is folded into the FF weights host-side (W1' = W1*g1,
b1' = b1 + W1@ln1_b, b2' = b2 + ln1_b). LN stats come from DVE
bn_stats/bn_aggr; z*g1 is injected into the FF2 PSUM group by an
identity matmul so LN2 reads stats/z straight from PSUM. The FF runs as
two row-tile sweeps (t=0 first, so ReduceScatter #2 hides under it; the
t=1 sweep walks w1/w2 blocks in reverse so the freshest blocks are
still resident), with FF2 trailing FF1 by one mi to hide relu latency.

Queue/engine discipline (the scheduling model serializes each DMA queue
and parks it on the first waiting transfer):
- SP: input prefetch stream + oblique reads (self-pacing WAR parks)
- Act HWDGE: B writes, attn_part writes, scores-time weight chunks
  (512KB chunks so they never monopolize the shared DMA pool)
- Pool SWDGE: collectives + rsx readbacks ONLY (a collective holds its
  SEQ while waiting, so nothing else may queue behind it)
- bd casts: DVE (+Pool for the first half, whose queue is still clear);
  exp is the Act-engine pacer so everything else avoids Act during the
  second half. Act Sqrt (LN rstd) is emitted only after the attn_part
  writes so its table switch cannot delay the ReduceScatter.
"""
import functools
from contextlib import ExitStack
import numpy as np

QLEN, MLEN, BSZ = 1024, 1024, 2
KLEN = QLEN + MLEN
D, H, DH, DI = 1024, 16, 64, 4096
HPG = 4                      # heads per group (per core)
HD_G = HPG * DH              # 256
N_CORES = 8
SCALE = 1.0 / (DH ** 0.5)
NEG = -1e30
BW = 2176                    # padded DRAM width for B (>= 2175)
NQT = QLEN // 128            # 8 query tiles of 128
NKT = KLEN // 128            # 16 key tiles of 128
NDC = D // 128               # 8 d-chunks
NMI = DI // 128              # 32 inner tiles
ROWS = QLEN // 4             # 256 rows per core after RS


@functools.lru_cache(maxsize=2)
def _build(single_sim=False):
    import concourse.bacc as bacc
    import concourse.mybir as mybir
    import concourse.tile as tile
    from concourse import masks
    import bass_rust

    F32 = mybir.dt.float32
    BF16 = mybir.dt.bfloat16
    AF = mybir.ActivationFunctionType
    ALU = mybir.AluOpType

    nc = bacc.Bacc("TRN2", target_bir_lowering=False, debug=False,
                   num_devices=N_CORES)

    def din(name, shape, dt=F32):
        return nc.dram_tensor(name, shape, dt, kind="ExternalInput")

    cat_fm = din("cat_fm", [D, KLEN], BF16)   # [d, mems||w tokens], this b
    r_fm = din("r_fm", [D, KLEN], BF16)       # r transposed
    wpk = din("wpk", [D, 4 * HD_G], BF16)     # [wkT | wqT | wrT | wvT]
    biases = din("biases", [128, 8])          # raw rwb/rrb + pre-scaled
    woT = din("woT", [HD_G, D], BF16)         # W_o^T rows for group
    w1T = din("w1T", [D, DI], BF16)           # (W1 * g1)^T
    b1c = din("b1c", [128, NMI])              # b1' packed column-wise
    w2T = din("w2T", [DI, D], BF16)
    b2s = din("b2s", [1, D], BF16)            # b2 + ln1_b, single row
    rows4 = din("rows4", [128, 3 * D], BF16)  # [g1 | g2 | ln2_b] bcast rows
    wres2 = din("wres2", [128, 2 * D], BF16)  # w rows for residual (2 tiles)

    Bh = [nc.dram_tensor(f"Bh{h}", [QLEN * BW], BF16) for h in range(HPG)]
    if single_sim:
        attn_part = nc.dram_tensor("attn_part", [QLEN, D], BF16,
                                   kind="ExternalOutput")
    else:
        attn_part = nc.dram_tensor("attn_part", [QLEN, D], BF16)
    rs_out = nc.dram_tensor("rs_out", [ROWS, D], BF16)
    y = nc.dram_tensor("y", [ROWS, D], F32, kind="ExternalOutput")

    def obl(h, qh2, ktb):
        # oblique view of Bh[h]: BD tiles [query, key] for 4 q-tiles x 512
        # keys at (q-tiles 4*qh2.., keys 512*ktb..); BD[i,j] = B[i, j-i+1023];
        # matches dest [128, 4, 512]
        off = 1023 + 512 * qh2 * (BW - 1) + 512 * ktb
        return bass_rust.AP(tensor=Bh[h].ap().tensor, offset=off,
                            ap=[[BW - 1, 128], [128 * (BW - 1), 4], [1, 512]])

    def bwrite(h, qt, c0, w):
        # B row tile [128, w] at (row 128*qt, col c0)
        off = 128 * qt * BW + c0
        return bass_rust.AP(tensor=Bh[h].ap().tensor, offset=off,
                            ap=[[BW, 128], [1, w]])

    def bpad(h):
        # all pad columns of head h as one flat write; src is a [128, 128]
        # NEG tile re-read NQT times via a stride-0 leading dim
        off = 2048
        return bass_rust.AP(tensor=Bh[h].ap().tensor, offset=off,
                            ap=[[128 * BW, NQT], [BW, 128], [1, BW - 2048]])

    with tile.TileContext(nc) as tc:
        with tc.tile_pool(name="const", bufs=1) as cpool, \
             tc.tile_pool(name="work", bufs=2) as wpool, \
             tc.tile_pool(name="psA", bufs=2, space="PSUM") as psA, \
             tc.tile_pool(name="psB", bufs=2, space="PSUM") as psB, \
             tc.tile_pool(name="psBB", bufs=2, space="PSUM") as psBB, \
             tc.tile_pool(name="psVT", bufs=2, space="PSUM") as psVT:
            w1pool = w2pool = None  # opened after the P1 scope closes

            # ---------------- global constants ----------------
            identb = cpool.tile([128, 128], BF16, tag="identb")
            masks.make_identity(nc, identb[:])
            ident = cpool.tile([128, 128], F32, tag="ident")
            masks.make_identity(nc, ident[:])
            bias_t = cpool.tile([128, 8], F32, tag="bias")
            nc.scalar.dma_start(out=bias_t[:], in_=biases[:])
            b1c_t = cpool.tile([128, NMI], F32, tag="b1c")
            nc.scalar.dma_start(out=b1c_t[:], in_=b1c[:])
            # z (normalized LN1 output) survives into FF scope
            zt = [cpool.tile([128, D], BF16, tag=f"zt_{t}", name=f"zt_{t}")
                  for t in range(ROWS // 128)]
            out1_fm = [None] * NDC
            zgs = [None] * (ROWS // 128)

            onesr = cpool.tile([1, 128], BF16, tag="onesr")
            nc.vector.memset(onesr[:], 1.0)
            b2b = cpool.tile([1, D], BF16, tag="b2b")
            nc.scalar.dma_start(out=b2b[:], in_=b2s[:])
            zpad = cpool.tile([128, BW - 2048], BF16, tag="zpad")
            nc.vector.memset(zpad[:], NEG)

            # FF prefetch destinations (loads issued mid-attention)
            rows1_t = cpool.tile([128, D], BF16, tag="rows1")
            wres_t = cpool.tile([128, 2 * D], BF16, tag="wres2")
            rsx_t = cpool.tile([128, 2 * D], BF16, tag="rsx")

            # w1/w2 as 4 big block tiles each, one batched DMA per block
            # (3-dim src AP, k-chunks side by side in the free dim)
            w1big = [None] * 4
            w2big = [None] * 4

            # weight-block loads are CHUNKED (4 x 512KB) so they interleave
            # with latency-critical small transfers on the shared DMA pool.
            # During scores they ride the Act queue (no WAR -> never park);
            # FF-time reloads use SP (their WAR parks are harmless there).
            def load_w1_block(blk, eng=None):
                eng = eng or nc.sync
                bt = w1pool.tile([128, 8192], BF16, tag="w1big",
                                 bufs=3, name=f"w1big{blk}")
                for c in range(4):
                    src = bass_rust.AP(
                        tensor=w1T.ap().tensor,
                        offset=1024 * (blk % 4) + 2 * c * 128 * DI,
                        ap=[[DI, 128], [128 * DI, 2], [1, 1024]])
                    eng.dma_start(out=bt[:, 2048 * c:2048 * c + 2048],
                                  in_=src)
                w1big[blk % 4] = bt

            def load_w2_block(blk, eng=None):
                eng = eng or nc.sync
                bt = w2pool.tile([128, 8192], BF16, tag="w2big",
                                 bufs=2, name=f"w2big{blk}")
                for c in range(4):
                    src = bass_rust.AP(
                        tensor=w2T.ap().tensor,
                        offset=(8 * (blk % 4) + 2 * c) * 128 * D,
                        ap=[[D, 128], [128 * D, 2], [1, 1024]])
                    eng.dma_start(out=bt[:, 2048 * c:2048 * c + 2048],
                                  in_=src)
                w2big[blk % 4] = bt

            # shared attention tensors (outlive the P1 scope)
            woT_t = []
            for k in range(HD_G // 128):
                tt = cpool.tile([128, D], BF16, tag=f"woT{k}", name=f"woT{k}")
                woT_t.append(tt)
            k_fm, rk_fm, qh_fm = [], [], []
            for m in range(2):
                k_fm.append(cpool.tile([128, KLEN], BF16, tag=f"kfm{m}",
                                       name=f"kfm{m}"))
                rk_fm.append(cpool.tile([128, KLEN], BF16, tag=f"rkfm{m}",
                                        name=f"rkfm{m}"))
                qh_fm.append(cpool.tile([128, QLEN], BF16, tag=f"qhfm{m}",
                                        name=f"qhfm{m}"))
            v_tok = []
            for kt in range(NKT):
                v_tok.append(cpool.tile([128, 65 * HPG], BF16, tag=f"vtok{kt}",
                                        name=f"vtok{kt}"))
            qt_fm = []
            for m in range(2):
                qt_fm.append(cpool.tile([128, QLEN], BF16, tag=f"qtfm{m}",
                                        name=f"qtfm{m}"))
            vecT_fm = {}
            for m in range(2):
                for hf in range(2):
                    vecT_fm[(m, hf)] = cpool.tile(
                        [128, QLEN // 2], BF16, tag=f"vecT{m}_{hf}",
                        name=f"vecT{m}_{hf}")

            # ================ P1 scope (cat/pw/qt/r die with it) ========
            with tc.tile_pool(name="p1", bufs=1) as apool:

                # ---------------- P1: inputs then projection weights ------
                # cat/pw/r as big sliced tiles; loads stream in consumption
                # order as a few batched 3-dim-AP DMAs on the SP queue
                catb = apool.tile([128, NDC * KLEN], BF16, tag="catb",
                                  name="catb")
                pwb = apool.tile([128, NDC * 4 * HD_G], BF16, tag="pwb",
                                 name="pwb")
                rb = apool.tile([128, NDC * KLEN], BF16, tag="rb", name="rb")
                cat_t = [catb[:, KLEN * k:KLEN * k + KLEN] for k in range(NDC)]
                pw = [pwb[:, 1024 * k:1024 * k + 1024] for k in range(NDC)]
                r_t = [rb[:, KLEN * k:KLEN * k + KLEN] for k in range(NDC)]

                def cat_blk(c0, w):
                    dst = bass_rust.AP(
                        tensor=catb[:].tensor, offset=c0,
                        ap=[[NDC * KLEN, 128], [KLEN, NDC], [1, w]])
                    src = bass_rust.AP(
                        tensor=cat_fm.ap().tensor, offset=c0,
                        ap=[[KLEN, 128], [128 * KLEN, NDC], [1, w]])
                    nc.sync.dma_start(out=dst, in_=src)

                def pw_part(c0, w):
                    dst = bass_rust.AP(
                        tensor=pwb[:].tensor, offset=c0,
                        ap=[[NDC * 1024, 128], [1024, NDC], [1, w]])
                    src = bass_rust.AP(
                        tensor=wpk.ap().tensor, offset=c0,
                        ap=[[1024, 128], [128 * 1024, NDC], [1, w]])
                    nc.sync.dma_start(out=dst, in_=src)

                pw_part(3 * HD_G, HD_G)      # wv first (V proj gate)
                cat_blk(0, 256)              # V kt0-1 gate: small first chunk
                cat_blk(256, 256)
                pw_part(0, 2 * HD_G)         # wk | wq (K starts early)
                cat_blk(512, 512)
                cat_blk(1024, 512)
                cat_blk(1536, 512)
                pw_part(2 * HD_G, HD_G)      # wr
                rsrc = bass_rust.AP(
                    tensor=r_fm.ap().tensor, offset=0,
                    ap=[[KLEN, 128], [128 * KLEN, NDC], [1, KLEN]])
                nc.sync.dma_start(out=rb[:], in_=rsrc)
                for k in range(HD_G // 128):
                    nc.sync.dma_start(out=woT_t[k][:],
                                      in_=woT[128 * k:128 * k + 128, :])

                def pw_sl(k, which, m):
                    base = {"wkT": 0, "wqT": 1, "wrT": 2, "wvT": 3}[which] * HD_G
                    return pw[k][:, base + 128 * m:base + 128 * m + 128]

                # V proj (token-major, interleaved ones cols) and K proj,
                # interleaved so each only needs the cat column blocks that
                # have landed (V kt-tiles pace with cat_blk arrivals, K rides
                # in between)
                def v_tile(kt):
                    vt = v_tok[kt]
                    ps = psB.tile([128, HD_G], F32, tag="psB", name="psv")
                    for k in range(NDC):
                        nc.tensor.matmul(
                            ps[:], cat_t[k][:, 128 * kt:128 * kt + 128],
                            pw[k][:, 3 * HD_G:4 * HD_G],
                            start=(k == 0), stop=(k == NDC - 1))
                    for hh in range(HPG):
                        nc.scalar.activation(vt[:, 65 * hh:65 * hh + 64],
                                             ps[:, 64 * hh:64 * hh + 64],
                                             AF.Copy)
                        nc.vector.memset(vt[:, 65 * hh + 64:65 * hh + 65], 1.0)

                def k_tile(m, n):
                    ps = psA.tile([128, 512], F32, tag="psA", name="psk")
                    for k in range(NDC):
                        nc.tensor.matmul(
                            ps[:], pw_sl(k, "wkT", m),
                            cat_t[k][:, 512 * n:512 * n + 512],
                            start=(k == 0), stop=(k == NDC - 1))
                    nc.vector.tensor_copy(
                        k_fm[m][:, 512 * n:512 * n + 512], ps[:])

                for kt in range(4):
                    v_tile(kt)
                k_tile(0, 0)
                k_tile(1, 0)
                for kt in range(4, 8):
                    v_tile(kt)
                k_tile(0, 1)
                k_tile(1, 1)
                for kt in range(8, 12):
                    v_tile(kt)
                k_tile(0, 2)
                k_tile(1, 2)
                for kt in range(12, 16):
                    v_tile(kt)
                k_tile(0, 3)
                k_tile(1, 3)
                # Q proj (cat cols MLEN.. only; last cat consumer)
                for m in range(2):
                    for n in range(QLEN // 512):
                        ps = psA.tile([128, 512], F32, tag="psA", name="psq")
                        for k in range(NDC):
                            nc.tensor.matmul(
                                ps[:], pw_sl(k, "wqT", m),
                                cat_t[k][:, MLEN + 512 * n:MLEN + 512 * n + 512],
                                start=(k == 0), stop=(k == NDC - 1))
                        # (q + bias) * SCALE on DVE, cast to bf16
                        nc.vector.tensor_scalar(
                            out=qh_fm[m][:, 512 * n:512 * n + 512], in0=ps[:],
                            scalar1=bias_t[:, m:m + 1], scalar2=SCALE,
                            op0=ALU.add, op1=ALU.mult)
                        nc.scalar.activation(
                            qt_fm[m][:, 512 * n:512 * n + 512], ps[:],
                            AF.Identity, scale=SCALE,
                            bias=bias_t[:, 6 + m:7 + m])
                # NEG pad columns of B (only needed before first oblique read)
                zsrc = bass_rust.AP(
                    tensor=zpad[:].tensor, offset=zpad[:].offset,
                    ap=[[BW - 2048, 128], [0, NQT], [1, BW - 2048]])
                for h in range(HPG):
                    nc.scalar.dma_start(out=bpad(h), in_=zsrc)
                for m in range(2):
                    for n in range(KLEN // 512):
                        ps = psA.tile([128, 512], F32, tag="psA", name="psr")
                        for k in range(NDC):
                            nc.tensor.matmul(
                                ps[:], pw_sl(k, "wrT", m),
                                r_t[k][:, 512 * n:512 * n + 512],
                                start=(k == 0), stop=(k == NDC - 1))
                        nc.vector.tensor_copy(
                            rk_fm[m][:, 512 * n:512 * n + 512], ps[:])

                # ---------------- P2: attention per head ----------------
                def head_slices(h):
                    m, p0 = h // 2, 64 * (h % 2)
                    return (m, p0, qh_fm[m][p0:p0 + 64, :],
                            qt_fm[m][p0:p0 + 64, :],
                            k_fm[m][p0:p0 + 64, :],
                            rk_fm[m][p0:p0 + 64, :])

                def b_phase(h):
                    m, p0, qh_h, qt_h, k_h, rk_h = head_slices(h)
                    # B = q_tilde @ rk^T -> f32 PSUM -> SBUF bf16 -> one DMA
                    # per q-tile; for qt<=3 the leading 512 cols are never
                    # read back -> skip them. Writes go on the Activation
                    # queue so oblique reads aren't queued behind them.
                    for g in range(NQT // 2):
                        # exact causal band: rows of q-tile qt only ever read
                        # back columns c >= 896 - 128*qt; a qt PAIR shares
                        # one staging tile and ONE batched 3-dim-AP write
                        # (halves the SP trigger count in the hot window)
                        bs = wpool.tile([128, 2 * KLEN], BF16, tag="big",
                                        bufs=2, name=f"bs{h}_{g}")
                        for qi in range(2):
                            qt = 2 * g + qi
                            c0 = max(0, 896 - 128 * qt)
                            for ct in range(c0 // 512, KLEN // 512):
                                lo = max(c0, 512 * ct)
                                wdt = 512 * (ct + 1) - lo
                                pb = psBB.tile([128, 512], F32, tag="psBB",
                                               name="psb")
                                nc.tensor.matmul(
                                    pb[:, 0:wdt],
                                    qt_h[:, 128 * qt:128 * qt + 128],
                                    rk_h[:, lo:lo + wdt],
                                    start=True, stop=True)
                                sl = bs[:, KLEN * qi + lo:KLEN * qi + lo + wdt]
                                if (h + qt + ct) % 2 == 0:
                                    nc.vector.tensor_copy(sl, pb[:, 0:wdt])
                                else:
                                    nc.scalar.activation(sl, pb[:, 0:wdt],
                                                         AF.Copy)
                        # write both q-tiles from the pair's common column
                        # start (the extra columns of the earlier qt are
                        # never read back)
                        cmin = max(0, 896 - 128 * (2 * g + 1))
                        w = KLEN - cmin
                        dst = bass_rust.AP(
                            tensor=Bh[h].ap().tensor,
                            offset=128 * 2 * g * BW + cmin,
                            ap=[[BW, 128], [128 * BW, 2], [1, w]])
                        srcap = bass_rust.AP(
                            tensor=bs[:].tensor, offset=bs[:].offset + cmin,
                            ap=[[2 * KLEN, 128], [KLEN, 2], [1, w]])
                        nc.sync.dma_start(out=dst, in_=srcap)

                def scores(h, qh2):
                    # key-major scores: S^T[key, q] = AC^T + BD^T; BD comes
                    # back from DRAM per-ktb [128, 4x512] bf16, cast to f32
                    # on DVE/Pool (SBUF->SBUF; Pool cannot touch PSUM), then
                    # PE-transpose-accumulated into the AC^T psum group
                    m, p0, qh_h, qt_h, k_h, rk_h = head_slices(h)
                    probT = [wpool.tile([128, 512], BF16, tag=f"pT{kt}",
                                        bufs=1, name=f"pT{kt}_{h}_{qh2}")
                             for kt in range(NKT)]
                    nktb = 3 if qh2 == 0 else 4
                    bd_t = {}
                    for ktb in range(nktb):
                        bd16 = wpool.tile([128, 2048], BF16, tag="bd16",
                                          bufs=4, name=f"bd16_{ktb}")
                        # oblique reads on SP: by scores time the prefetch
                        # stream has drained, so their WAR parks self-pace
                        nc.sync.dma_start(out=bd16[:], in_=obl(h, qh2, ktb))
                        for qi, qt in enumerate(range(4 * qh2, 4 * qh2 + 4)):
                            kmax = min(qt + 8, NKT - 1)
                            wdt = min(512, (kmax + 1 - 4 * ktb) * 128)
                            if wdt <= 0:
                                continue
                            bd = wpool.tile([128, 512], F32, tag="bd",
                                            bufs=5, name=f"bd{qt}_{ktb}")
                            # cast engines: first half can use Pool (its
                            # queue is clear until collective(0) is reached);
                            # second half must avoid Pool (collective parks
                            # its SEQ) so Act takes the overflow
                            if qh2 == 0 and (qi + ktb) % 2 == 0:
                                eng = nc.gpsimd
                            else:
                                eng = nc.vector
                            if eng is nc.scalar:
                                eng.activation(bd[:, 0:wdt],
                                               bd16[:, 512 * qi:512 * qi + wdt],
                                               AF.Copy)
                            else:
                                eng.tensor_copy(bd[:, 0:wdt],
                                                bd16[:, 512 * qi:512 * qi + wdt])
                            bd_t[(qt, ktb)] = bd
                    for kt in range(4 * nktb):
                        qts = [qt for qt in range(4 * qh2, 4 * qh2 + 4)
                               if qt >= kt - 8]
                        if not qts:
                            continue
                        if qh2 == 1:
                            pspool, pstag = [(psA, "psA"), (psB, "psB"),
                                             (psBB, "psBB")][kt % 3]
                        else:
                            pspool, pstag = ((psA, "psA") if kt % 2 == 0
                                             else (psB, "psB"))
                        ps = pspool.tile([128, 512], F32, tag=pstag,
                                         name="pss")
                        sub0 = 128 * (qts[0] - 4 * qh2)
                        w0 = 128 * (qts[-1] + 1 - qts[0])
                        nc.tensor.matmul(
                            ps[:, sub0:sub0 + w0],
                            k_h[:, 128 * kt:128 * kt + 128],
                            qh_h[:, 512 * qh2 + sub0:512 * qh2 + sub0 + w0],
                            start=True, stop=False)
                        for i, qt in enumerate(qts):
                            bd = bd_t[(qt, kt // 4)]
                            bo = 128 * (kt % 4)
                            sub = 128 * (qt - 4 * qh2)
                            nc.tensor.matmul(ps[:, sub:sub + 128],
                                             bd[:, bo:bo + 128],
                                             ident[:], is_transpose=True,
                                             start=False,
                                             stop=(i == len(qts) - 1))
                        blo = qts[0]
                        sub = 128 * (blo - 4 * qh2)
                        w = 128 * (4 * qh2 + 4 - blo)
                        nc.scalar.activation(probT[kt][:, sub:sub + w],
                                             ps[:, sub:sub + w], AF.Exp)
                    for qt in range(4 * qh2, 4 * qh2 + 4):
                        vec_qt(h, qh2, qt, probT)

                def vec_qt(h, qh2, qt, probT):
                    m, p0 = h // 2, 64 * (h % 2)
                    kmax = min(qt + 8, NKT - 1)
                    pv = psVT.tile([128, 65], F32, tag="psVT", name="pv")
                    sub = 128 * (qt - 4 * qh2)
                    for kt in range(kmax + 1):
                        nc.tensor.matmul(
                            pv[:], probT[kt][:, sub:sub + 128],
                            v_tok[kt][:, 65 * h:65 * h + 65],
                            start=(kt == 0), stop=(kt == kmax))
                    rec = wpool.tile([128, 1], F32, tag="rec", name="rec")
                    nc.vector.reciprocal(rec[:], pv[:, 64:65])
                    vn = wpool.tile([128, 64], BF16, tag="vn", name="vn")
                    nc.vector.tensor_scalar_mul(vn[:], pv[:, 0:64], rec[:])
                    pt = psVT.tile([64, 128], BF16, tag="psVT", name="ptr")
                    nc.tensor.matmul(pt[:], vn[:], identb[:],
                                     is_transpose=True,
                                     start=True, stop=True)
                    dst = vecT_fm[(m, qh2)][p0:p0 + 64,
                                            128 * (qt % 4):128 * (qt % 4) + 128]
                    if qh2 == 1:
                        nc.vector.tensor_copy(dst, pt[:])
                    else:
                        nc.scalar.activation(dst, pt[:], AF.Copy)

                def p3_half(qh2):
                    # partial attn_out for this half's 4 q-tiles, then the
                    # quad ReduceScatter for it and the rs_out readback
                    for qg in range(2 * qh2, 2 * qh2 + 2):
                        ao = wpool.tile([128, KLEN], BF16, tag="big",
                                        bufs=2, name=f"ao{qg}")
                        for qi in range(2):
                            qt = 2 * qg + qi
                            for n in range(D // 512):
                                ps = psA.tile([128, 512], F32, tag="psA",
                                              name="pso")
                                for k in range(2):
                                    nc.tensor.matmul(
                                        ps[:],
                                        vecT_fm[(k, qt // 4)][:, 128 * (qt % 4):
                                                              128 * (qt % 4) + 128],
                                        woT_t[k][:, 512 * n:512 * n + 512],
                                        start=(k == 0), stop=(k == 1))
                                sl = ao[:, D * qi + 512 * n:
                                        D * qi + 512 * n + 512]
                                # DVE both halves: it is idle by p3 time,
                                # and the staging must not queue behind the
                                # Act exp backlog (it gates the collective)
                                nc.vector.tensor_copy(sl, ps[:])
                        # both q-tiles in one Act-queue write (staging Act
                        # copy precedes it in-queue; SP stream untouched)
                        dst = bass_rust.AP(
                            tensor=attn_part.ap().tensor, offset=256 * qg * D,
                            ap=[[D, 128], [128 * D, 2], [1, D]])
                        nc.sync.dma_start(out=dst, in_=ao[:, 0:2 * D])
                    s = qh2
                    if single_sim:
                        nc.sync.dma_start(
                            out=rs_out[128 * s:128 * s + 128, :],
                            in_=attn_part[512 * s:512 * s + 128, :])
                    else:
                        nc.gpsimd.collective_compute(
                            "ReduceScatter", ALU.add,
                            replica_groups=[[0, 1, 2, 3], [4, 5, 6, 7]],
                            ins=[attn_part[512 * s:512 * s + 512, :]],
                            outs=[rs_out[128 * s:128 * s + 128, :]])
                    if s == 1:
                        # readback(1) on the Pool SWDGE queue right behind
                        # its collective (SP would park ahead of the FF
                        # sweep's weight reloads)
                        nc.gpsimd.dma_start(
                            out=rsx_t[:, D:2 * D],
                            in_=rs_out[128:256, :])

                g1r = rows1_t[:, 0:D]

                def ln_normalize(x_t, z_out, nm):
                    # DVE-only LN via bn_stats/bn_aggr (two 512-halves), then
                    # z = rstd*x - m*rstd via one tensor_scalar pass
                    st = wpool.tile([128, 12], F32, tag="lnst", name=f"lnst{nm}")
                    nc.vector.bn_stats(st[:, 0:6], x_t[:, 0:512])
                    nc.vector.bn_stats(st[:, 6:12], x_t[:, 512:1024])
                    mv = wpool.tile([128, 2], F32, tag="lnmv", name=f"lnmv{nm}")
                    nc.vector.bn_aggr(mv[:], st[:])
                    ve = wpool.tile([128, 1], F32, tag="lnve", name=f"lnve{nm}")
                    nc.vector.tensor_scalar(out=ve[:], in0=mv[:, 1:2],
                                            scalar1=1e-5, scalar2=0.0,
                                            op0=ALU.add, op1=ALU.add)
                    rc = wpool.tile([128, 1], F32, tag="lnrc", name=f"lnrc{nm}")
                    nc.vector.reciprocal(rc[:], ve[:])
                    rstd = wpool.tile([128, 1], F32, tag="lnrstd",
                                      name=f"lnrstd{nm}")
                    nc.scalar.activation(rstd[:], rc[:], AF.Sqrt)
                    nb = wpool.tile([128, 1], F32, tag="lnnb", name=f"lnnb{nm}")
                    nc.vector.tensor_tensor(out=nb[:], in0=mv[:, 0:1],
                                            in1=rstd[:], op=ALU.mult)
                    nc.vector.tensor_scalar_mul(nb[:], nb[:], -1.0)
                    nc.vector.tensor_scalar(out=z_out, in0=x_t[:],
                                            scalar1=rstd[:], scalar2=nb[:],
                                            op0=ALU.mult, op1=ALU.add)

                def ln1_tile(t):
                    nm = f"l1{t}"
                    xt = wpool.tile([128, D], F32, tag="xres", bufs=2,
                                    name=f"xres{t}")
                    st = wpool.tile([128, 12], F32, tag="lnst", name=f"lnst{nm}")
                    for hf in range(2):
                        sl = slice(512 * hf, 512 * hf + 512)
                        nc.vector.tensor_tensor(
                            out=xt[:, sl],
                            in0=wres_t[:, D * t + 512 * hf:D * t + 512 * hf + 512],
                            in1=rsx_t[:, D * t + 512 * hf:D * t + 512 * hf + 512],
                            op=ALU.add)
                        nc.vector.bn_stats(st[:, 6 * hf:6 * hf + 6], xt[:, sl])
                    mv = wpool.tile([128, 2], F32, tag="lnmv", name=f"lnmv{nm}")
                    nc.vector.bn_aggr(mv[:], st[:])
                    ve = wpool.tile([128, 1], F32, tag="lnve", name=f"lnve{nm}")
                    nc.vector.tensor_scalar(out=ve[:], in0=mv[:, 1:2],
                                            scalar1=1e-5, scalar2=0.0,
                                            op0=ALU.add, op1=ALU.add)
                    rc = wpool.tile([128, 1], F32, tag="lnrc", name=f"lnrc{nm}")
                    nc.vector.reciprocal(rc[:], ve[:])
                    rstd = wpool.tile([128, 1], F32, tag="lnrstd",
                                      name=f"lnrstd{nm}")
                    nc.scalar.activation(rstd[:], rc[:], AF.Sqrt)
                    nb = wpool.tile([128, 1], F32, tag="lnnb", name=f"lnnb{nm}")
                    nc.vector.tensor_tensor(out=nb[:], in0=mv[:, 0:1],
                                            in1=rstd[:], op=ALU.mult)
                    nc.vector.tensor_scalar_mul(nb[:], nb[:], -1.0)
                    for hf in range(2):
                        sl = slice(512 * hf, 512 * hf + 512)
                        nc.vector.tensor_scalar(out=zt[t][:, sl], in0=xt[:, sl],
                                                scalar1=rstd[:], scalar2=nb[:],
                                                op0=ALU.mult, op1=ALU.add)

                def zT_zg(t):
                    # transpose z to feature-major + z*g1 residual staging
                    for k in range(NDC):
                        pt = psBB.tile([128, 128], BF16, tag="psBB",
                                       name=f"ptf{t}_{k}")
                        nc.tensor.matmul(pt[:], zt[t][:, 128 * k:128 * k + 128],
                                         identb[:], is_transpose=True,
                                         start=True, stop=True)
                        nc.scalar.activation(out1_fm[k][:, 128 * t:128 * t + 128],
                                             pt[:], AF.Copy)
                    nc.vector.tensor_tensor(out=zgs[t][:], in0=zt[t][:], in1=g1r,
                                            op=ALU.mult)

                # B per head pipelined ahead of its scores pair (probT and
                # all scores inputs live outside this scope, so scores may
                # interleave here; p1 closes after the last b_phase)
                b_phase(0)
                b_phase(1)
                scores(0, 0)
                b_phase(2)
                scores(1, 0)
                b_phase(3)

            # ====== FF-weight pools open once the P1 SBUF is recycled ====
            _es = ExitStack()
            w1pool = _es.enter_context(tc.tile_pool(name="w1p", bufs=1))
            w2pool = _es.enter_context(tc.tile_pool(name="w2p", bufs=1))
            # prefetch all FF weights on the Act queue: no deps, no parks,
            # and the SP stream (B writes + obliques) keeps self-pacing
            nc.scalar.dma_start(out=rows1_t[:], in_=rows4[:, 0:D])
            nc.scalar.dma_start(out=wres_t[:], in_=wres2[:])
            scores(2, 0)
            scores(3, 0)
            p3_half(0)
            # block-0 loads trigger here: the first-half window is DMA-pool
            # saturated (B writes + obliques); this one has plenty of slack
            load_w1_block(0, nc.scalar)
            load_w2_block(0, nc.scalar)
            scores(0, 1)
            scores(1, 1)
            scores(2, 1)
            scores(3, 1)
            # block-1 loads here: out of the DMA-saturated scores window,
            # still well ahead of FF sweep-0 mi=8
            load_w1_block(1, nc.scalar)
            load_w2_block(1, nc.scalar)
            # readback(0) on SP: every oblique is already queued, so its
            # park (until RS(0) lands) blocks nothing time-critical
            nc.sync.dma_start(out=rsx_t[:, 0:D], in_=rs_out[0:128, :])
            p3_half(1)           # issues RS(1); staging is Act-only
            ln1_tile(0)          # after p3: its Act Sqrt can't delay the
                                 # attn writes that gate RS(1)

            # ================ FF scope ================
            # t-split sweeps: all of t=0's FF1+FF2 first (independent of
            # RS(1)), so the second ReduceScatter hides under it; LN2(t0)
            # hides under the t=1 sweep; only LN2(t1) is an exposed tail.
            with tc.tile_pool(name="ff", bufs=1) as fpool:
                # FF staging piggybacks on wpool tags that are dead once
                # scores complete (probT / bd16), instead of new SBUF
                for k in range(NDC):
                    out1_fm[k] = wpool.tile([128, ROWS], BF16, tag=f"pT{k}",
                                            bufs=1, name=f"o1fm{k}")
                for t in range(ROWS // 128):
                    zgs[t] = wpool.tile([128, D], BF16, tag="bd16",
                                        bufs=4, name=f"zg{t}")
                rows23_t = wpool.tile([128, 2 * D], BF16, tag="bd16",
                                      bufs=4, name="rows23")
                nc.sync.dma_start(out=rows23_t[:], in_=rows4[:, D:3 * D])
                g2r = rows23_t[:, 0:D]
                lb2r = rows23_t[:, D:2 * D]

                hps = {}
                hps[(0, 0)] = psB.tile([128, 512], F32, tag="psB", name="h2ps00")
                hps[(0, 1)] = psB.tile([128, 512], F32, tag="psB", name="h2ps01")
                hps[(1, 0)] = psVT.tile([128, 512], F32, tag="psVT", name="h2ps10")
                hps[(1, 1)] = psVT.tile([128, 512], F32, tag="psVT", name="h2ps11")
                # inject b2 + ln1_b into each h2 group BEFORE the RS-gated
                # zT(0): out[p, j] += 1 * b2[j]
                for (t, n), hp in hps.items():
                    nc.tensor.matmul(hp[:], onesr[:, 0:128],
                                     b2b[:, 512 * n:512 * n + 512],
                                     start=True, stop=False)
                zT_zg(0)

                def ff1_mi(mi, t):
                    blk, mo = mi // 8, 128 * (mi % 8)
                    pool_, tag_ = (psA, "psA") if mi % 2 == 0 else (psBB, "psBB")
                    ps = pool_.tile([128, 128], F32, tag=tag_,
                                    name=f"psh1_{mi}_{t}")
                    for k in range(NDC):
                        nc.tensor.matmul(
                            ps[:], w1big[blk][:, 1024 * k + mo:1024 * k + mo + 128],
                            out1_fm[k][:, 128 * t:128 * t + 128],
                            start=(k == 0), stop=(k == NDC - 1))
                    ht = fpool.tile([128, 128], BF16, tag="h1T", bufs=3,
                                    name=f"h1T{mi}_{t}")
                    nc.scalar.activation(ht[:], ps[:], AF.Relu,
                                         bias=b1c_t[:, mi:mi + 1])
                    return ht

                def ff2_mi(mi, t, ht, last):
                    blk = mi // 8
                    for n in range(D // 512):
                        nc.tensor.matmul(
                            hps[(t, n)][:], ht[:],
                            w2big[blk][:, 1024 * (mi % 8) + 512 * n:
                                        1024 * (mi % 8) + 512 * n + 512],
                            start=False, stop=last)

                def ln2_tile(t):
                    # x2 = z*g1 + core + b2 + ln1_b lives ENTIRELY in the
                    # FF2 psum (zg injected by an identity matmul), so LN2
                    # reads stats and z straight from psum; y goes out in
                    # column halves so the first DMA launches early
                    nm = f"l2{t}"
                    st = wpool.tile([128, 12], F32, tag="lnst", name=f"lnst{nm}")
                    for n in range(D // 512):
                        nc.vector.bn_stats(st[:, 6 * n:6 * n + 6],
                                           hps[(t, n)][:])
                    mv = wpool.tile([128, 2], F32, tag="lnmv", name=f"lnmv{nm}")
                    nc.vector.bn_aggr(mv[:], st[:])
                    ve = wpool.tile([128, 1], F32, tag="lnve", name=f"lnve{nm}")
                    nc.vector.tensor_scalar(out=ve[:], in0=mv[:, 1:2],
                                            scalar1=1e-5, scalar2=0.0,
                                            op0=ALU.add, op1=ALU.add)
                    rc = wpool.tile([128, 1], F32, tag="lnrc", name=f"lnrc{nm}")
                    nc.vector.reciprocal(rc[:], ve[:])
                    rstd = wpool.tile([128, 1], F32, tag="lnrstd",
                                      name=f"lnrstd{nm}")
                    nc.scalar.activation(rstd[:], rc[:], AF.Sqrt)
                    nb = wpool.tile([128, 1], F32, tag="lnnb", name=f"lnnb{nm}")
                    nc.vector.tensor_tensor(out=nb[:], in0=mv[:, 0:1],
                                            in1=rstd[:], op=ALU.mult)
                    nc.vector.tensor_scalar_mul(nb[:], nb[:], -1.0)
                    z2 = wpool.tile([128, D], BF16, tag="bd", bufs=5,
                                    name=f"z2_{t}")
                    yt = wpool.tile([128, D], F32, tag="big", bufs=2,
                                    name=f"y_{t}")
                    for n in range(D // 512):
                        sl = slice(512 * n, 512 * n + 512)
                        nc.vector.tensor_scalar(out=z2[:, sl],
                                                in0=hps[(t, n)][:],
                                                scalar1=rstd[:], scalar2=nb[:],
                                                op0=ALU.mult, op1=ALU.add)
                        nc.vector.tensor_tensor(out=yt[:, sl], in0=z2[:, sl],
                                                in1=g2r[:, sl], op=ALU.mult)
                        nc.vector.tensor_tensor(out=yt[:, sl], in0=yt[:, sl],
                                                in1=lb2r[:, sl], op=ALU.add)
                        nc.sync.dma_start(
                            out=y[128 * t:128 * t + 128, 512 * n:512 * n + 512],
                            in_=yt[:, sl])

                # t=0 sweep walks w1/w2 blocks 0..3; the t=1 sweep runs in
                # REVERSE mi order so blocks 3,2 are still resident at the
                # turn and only 1,0 reload (hidden under FF compute).
                # FF2 trails FF1 by one mi so the relu latency never stalls
                # the PE queue.
                pend = None
                # zg(0) joins the hps(0,*) groups up front (group order is
                # free), keeping the groups' stop on the last FF2 matmul
                for n in range(D // 512):
                    nc.tensor.matmul(hps[(0, n)][:], identb[:],
                                     zgs[0][:, 512 * n:512 * n + 512],
                                     start=False, stop=False)
                for mi in range(NMI):
                    if mi == 0:
                        load_w1_block(2)
                        load_w2_block(2)
                    elif mi == 8:
                        load_w1_block(3)
                        load_w2_block(3)
                    elif mi == 24:
                        ln1_tile(1)      # DVE-only chain; parks idle DVE
                    ht = ff1_mi(mi, 0)
                    if pend is not None:
                        ff2_mi(*pend, last=False)
                    pend = (mi, 0, ht)
                ff2_mi(*pend, last=True)
                pend = None
                zT_zg(1)
                for n in range(D // 512):
                    nc.tensor.matmul(hps[(1, n)][:], identb[:],
                                     zgs[1][:, 512 * n:512 * n + 512],
                                     start=False, stop=False)
                for i, mi in enumerate(reversed(range(NMI))):
                    if i == 0:
                        # w1 blk1 is STILL RESIDENT (bufs=3); only w2 reloads
                        load_w2_block(5)
                    elif i == 8:
                        # advance w1 rotation past blk1's live slot, then
                        # reload blk0 into blk2's slot (free after i=15)
                        w1pool.tile([128, 8192], BF16, tag="w1big",
                                    bufs=3, name="w1skip")
                        load_w1_block(4)
                        load_w2_block(4)
                    elif i == 2:
                        ln2_tile(0)      # hps(0,·) complete; DVE/Pool only
                    ht = ff1_mi(mi, 1)
                    if pend is not None:
                        ff2_mi(*pend, last=False)
                    pend = (mi, 1, ht)
                ff2_mi(*pend, last=True)
                ln2_tile(1)
            _es.close()

    nc.compile()
    return nc


def _prep_inputs(w, r, mems, W_qkv, W_r, W_o, r_w_bias, r_r_bias,
                 ln1_g, ln1_b, ff_W1, ff_b1, ff_W2, ff_b2, ln2_g, ln2_b,
                 attn_mask=None):
    import ml_dtypes
    f32 = np.float32
    bf16 = ml_dtypes.bfloat16
    cat = np.concatenate([mems, w], axis=0)            # [KLEN, B, D]
    cat_fm = [np.ascontiguousarray(cat[:, b, :].T).astype(bf16)
              for b in range(BSZ)]
    r_fm = np.ascontiguousarray(r.T).astype(bf16)
    # LN1 affine folded into FF weights
    W1p = np.asarray(ff_W1, f32) * np.asarray(ln1_g, f32)[None, :]
    b1p = np.asarray(ff_b1, f32) + np.asarray(ff_W1, f32) @ np.asarray(ln1_b, f32)
    b2p = np.asarray(ff_b2, f32) + np.asarray(ln1_b, f32)
    w1T = np.ascontiguousarray(W1p.T).astype(bf16)     # [D, DI]
    w2T = np.ascontiguousarray(np.asarray(ff_W2, f32).T).astype(bf16)
    woT_full = np.ascontiguousarray(W_o.T, dtype=f32)  # [H*DH, D]
    b1c = np.ascontiguousarray(b1p.reshape(NMI, 128).T)  # [128, NMI]
    rowb = lambda v: np.broadcast_to(
        np.asarray(v, f32).reshape(1, D), (128, D))
    rows4 = np.ascontiguousarray(np.concatenate(
        [rowb(ln1_g), rowb(ln2_g), rowb(ln2_b)], axis=1)).astype(bf16)

    in_maps = []
    for c in range(N_CORES):
        b, g = c // 4, c % 4
        sl = slice(HD_G * g, HD_G * g + HD_G)
        wkT = np.asarray(W_qkv, f32)[H * DH:2 * H * DH][sl].T
        wqT = np.asarray(W_qkv, f32)[0:H * DH][sl].T
        wrT = np.asarray(W_r, f32)[sl].T
        wvT = np.asarray(W_qkv, f32)[2 * H * DH:3 * H * DH][sl].T
        wpk = np.concatenate([wkT, wqT, wrT, wvT], axis=1)  # [D, 4*HD_G]
        rwbv = np.asarray(r_w_bias, f32).reshape(-1)[sl]
        rrbv = np.asarray(r_r_bias, f32).reshape(-1)[sl]
        bias = np.stack([
            rwbv[0:128], rwbv[128:256], rrbv[0:128], rrbv[128:256],
            rwbv[0:128] * SCALE, rwbv[128:256] * SCALE,
            rrbv[0:128] * SCALE, rrbv[128:256] * SCALE,
        ], axis=1)                                          # [128, 8]
        wres2 = np.concatenate(
            [np.asarray(w, f32)[128 * g:128 * g + 128, b, :],
             np.asarray(w, f32)[512 + 128 * g:512 + 128 * g + 128, b, :]],
            axis=1)                                         # [128, 2*D]
        m = {
            "cat_fm": cat_fm[b],
            "r_fm": r_fm,
            "wpk": np.ascontiguousarray(wpk).astype(bf16),
            "biases": np.ascontiguousarray(bias),
            "woT": np.ascontiguousarray(woT_full[sl]).astype(bf16),
            "w1T": w1T, "b1c": b1c, "w2T": w2T,
            "b2s": np.ascontiguousarray(b2p.reshape(1, D)).astype(bf16),
            "rows4": rows4,
            "wres2": np.ascontiguousarray(wres2).astype(bf16),
        }
        in_maps.append(m)
    return in_maps


def kernel(**inputs):
    from concourse.bass_utils import run_bass_kernel_spmd
    nc = _build()
    in_maps = _prep_inputs(**{k: np.asarray(v) for k, v in inputs.items()})
    res = run_bass_kernel_spmd(nc, in_maps, list(range(N_CORES)))
    out = np.empty((QLEN, BSZ, D), np.float32)
    for c in range(N_CORES):
        b, g = c // 4, c % 4
        yv = res.results[c]["y"]
        out[128 * g:128 * g + 128, b, :] = yv[0:128]
        out[512 + 128 * g:512 + 128 * g + 128, b, :] = yv[128:256]
    return out

